# revision 47
# baseline (speedup 1.0000x reference)
"""Trainium2 Bass kernel for sliding-window causal attention (CausalAttention).

Computation (per reference):
    qkv = x @ W_qkv + b_qkv            # [BF, T, 3C]
    split into per-head q, k, v (head dim 64)
    scores = (q @ k.T) / sqrt(hd) + band_bias(pe)   # band: 0 <= i-j <= 31
    out = softmax(scores) @ v          # [BF, T, C]

Sharding: data-parallel over BF across 8 cores (8 bf rows per core).
Device strategy (per core, bf_l=8, tok=2048):
  - qkv projection in fp8e4m3 DoubleRow matmuls (3/4 the bf16 PE cycles,
    tighter than bf16 numerically) via the three-term split
    x@W = A@W1 + C@W2 + B@W3 with A=fp8(x), C=fp8(4(x-A)), B=fp8(x/16),
    W1=fp8(W), W2=fp8(W/4), W3=fp8(16(W-W1)).  All term planes are packed
    host-side into a sub-chunk-major fp8 image (xF) plus m-major q/k and
    j-major v weight images.  The 1/8 score scale rides the q evacuation
    (scaling weights before fp8 quantization would crush them into
    subnormals).
  - scores are computed transposed (ST[key, query]) per query-block window
    with 64-deep contractions using 64-row tile_position quadrants (even
    heads rows 0-63, odd heads 64-127), so q and k evacuate once each with
    no parity masks.  The PE faults on direct switches between different
    64-row quadrant configs, so every score matmul is preceded by a tiny
    full-128-row guard matmul that reads the same kT/qT tiles (identical
    readiness + WAW corner overlap keep the pair adjacent under the tile
    scheduler).
  - four heads share a PSUM bank per score block; exp runs on ACT, the
    exp(band-bias) multiply on DVE; the final qblock's 8 heads share a
    single bank/exp.
  - softmax denominators accumulate in a separate [128,16] PSUM tile via
    1-column ones-matmuls; the division is a reciprocal plus one
    scalar_tensor_tensor per output tile writing fp16 y (converted and
    v-bias-added on the host: softmax weights sum to 1, so the v bias is a
    per-channel constant).
  - emission is software-pipelined: projection octs and v-projections are
    drip-fed between score and AV stages (PE wait queue is 4 deep), and the
    final two bf interleave their stages out of the idle qkv PSUM pool.
"""

import os
import numpy as np
from contextlib import ExitStack

import concourse.bacc as bacc
import concourse.bass as bass
import concourse.tile as tile
import concourse.mybir as mybir
from concourse.bass_utils import run_bass_kernel_spmd

BF, T, C = 64, 256, 512
NH, LB, HD = 8, 31, 64
NCORES = 8
BFL = BF // NCORES            # 8 bf rows per core
TOK = BFL * T                 # 2048 tokens per core
NEG = -1.0e30

F32 = mybir.dt.float32
F32R = mybir.dt.float32r
F16 = mybir.dt.float16
BF16 = mybir.dt.bfloat16
F8 = mybir.dt.float8e4
AF = mybir.ActivationFunctionType
DR = mybir.MatmulPerfMode.DoubleRow

# query blocks: (q0, nq, k0, W)  -- window keys [k0, k0+W)
QBLOCKS = [
    (0, 96, 0, 96),
    (96, 96, 64, 128),
    (192, 64, 160, 96),
]
QB_OFF = [0, 768, 1536]       # column offset of each qblock in BT
# output pieces: (qb_idx, col_start_in_ex, nq_piece, psum_base==y_row, y_tile)
PIECES = [
    (0, 0, 96, 0, 0),
    (1, 0, 32, 96, 0),
    (1, 32, 64, 0, 1),
    (2, 0, 64, 64, 1),
]

# fp8 tri-term packing: per-partition strides inside xF / W images
#   xF layout  [128, s(8), j(6), kt(2), n(256)]  (j = term*2 + chpair)
#   Wq/Wk      [128, mb(4), j(6), kt(2), 128]    (m-major for startup DMAs)
#   Wv         [128, j(6), kt(2), m(512)]
XF_S = 6 * 2 * 256            # 3072
W_MB = 6 * 2 * 128            # 1536
W_J = 2 * 512                 # 1024

_CACHE = {}


def _build_module():
    nc = bacc.Bacc("TRN2", target_bir_lowering=False, debug=False,
                   num_devices=NCORES)

    dram = {}
    def din(name, shape, dt):
        dram[name] = nc.dram_tensor(name, shape, dt, kind="ExternalInput").ap()
    din("xF", [128, 8 * XF_S], F8)
    din("WqA", [128, 6 * W_J], F8)
    din("WkA", [128, 6 * W_J], F8)
    din("WvA", [128, 6 * W_J], F8)
    din("bq", [128, 4], F32)
    din("bk", [128, 4], F32)
    din("onec", [128, 1], BF16)
    din("EB", [128, 2048], BF16)
    din("ones", [1, 128], BF16)
    y_ap = nc.dram_tensor("y", [TOK, C], F16, kind="ExternalOutput").ap()

    NCH = max(1, TOK // 512)
    CW = TOK // NCH

    with tile.TileContext(nc) as tc:
        with ExitStack() as ctx:
            singles = ctx.enter_context(tc.tile_pool(name="singles", bufs=1))

            # ---- persistent SBUF tensors ----
            xF = singles.tile([128, 8 * XF_S], F8, tag="xF", name="xF")
            WqA = singles.tile([128, 6 * W_J], F8, tag="WqA", name="WqA")
            WkA = singles.tile([128, 6 * W_J], F8, tag="WkA", name="WkA")
            WvA = singles.tile([128, 6 * W_J], F8, tag="WvA", name="WvA")
            bq = singles.tile([128, 4], F32, tag="bq")
            bk = singles.tile([128, 4], F32, tag="bk")
            onec = singles.tile([128, 1], BF16, tag="onec")
            ones_r = singles.tile([1, 128], BF16, tag="ones_r")
            EB = singles.tile([128, 2048], BF16, tag="EB")
            qT = [singles.tile([128, TOK], BF16, tag=f"qT{i}", name=f"qT{i}")
                  for i in range(4)]
            kT = [singles.tile([128, TOK], BF16, tag=f"kT{i}", name=f"kT{i}")
                  for i in range(4)]

            def load_xf(sb):
                nc.sync.dma_start(
                    out=xF[:, XF_S * sb:XF_S * (sb + 1)],
                    in_=dram["xF"][:, XF_S * sb:XF_S * (sb + 1)])
            def load_wq(mb):
                nc.sync.dma_start(
                    out=WqA[:, W_MB * mb:W_MB * (mb + 1)],
                    in_=dram["WqA"][:, W_MB * mb:W_MB * (mb + 1)])
            def load_wk(mb):
                nc.sync.dma_start(
                    out=WkA[:, W_MB * mb:W_MB * (mb + 1)],
                    in_=dram["WkA"][:, W_MB * mb:W_MB * (mb + 1)])
            load_wq(0)
            load_xf(0)
            nc.sync.dma_start(out=bq, in_=dram["bq"])
            nc.sync.dma_start(out=bk, in_=dram["bk"])
            load_xf(1)
            load_wk(0)
            load_wq(1)
            load_wk(1)
            load_wq(2)
            load_wk(2)
            load_wq(3)
            load_wk(3)
            nc.sync.dma_start(out=WvA, in_=dram["WvA"])
            nc.sync.dma_start(out=onec, in_=dram["onec"])
            nc.sync.dma_start(out=EB[:, 0:768], in_=dram["EB"][:, 0:768])
            nc.sync.dma_start(out=ones_r, in_=dram["ones"])
            load_xf(2)
            load_xf(3)
            nc.sync.dma_start(out=EB[:, 768:2048], in_=dram["EB"][:, 768:2048])
            for sb in range(4, 8):
                load_xf(sb)
            pqkv = ctx.enter_context(
                tc.tile_pool(name="psum_qkv", bufs=3, space="PSUM"))
            pst = ctx.enter_context(
                tc.tile_pool(name="psum_st", bufs=2, space="PSUM"))
            poa = ctx.enter_context(
                tc.tile_pool(name="psum_oa", bufs=2, space="PSUM"))
            pden = ctx.enter_context(
                tc.tile_pool(name="psum_den", bufs=1, space="PSUM"))
            vpool = ctx.enter_context(tc.tile_pool(name="vsb", bufs=10))
            epool = ctx.enter_context(tc.tile_pool(name="esb", bufs=16))
            rpool = ctx.enter_context(tc.tile_pool(name="rsb", bufs=8))
            ypool = ctx.enter_context(tc.tile_pool(name="ysb", bufs=8))

            def x_rhs(sb, j, w0, n):
                # xF[:, sb, j, :, w0:w0+n]  (kt-major pair for DoubleRow)
                return bass.AP(
                    tensor=xF.tensor,
                    offset=xF.offset + XF_S * sb + 512 * j + w0,
                    ap=[xF.ap[0], [256, 2], [1, n]])

            def wqk_lhs(Wt, o, j):
                # Wq/Wk[:, o, j, :, :]  (m-major image, 128-wide out tile)
                return bass.AP(
                    tensor=Wt.tensor,
                    offset=Wt.offset + W_MB * o + 256 * j,
                    ap=[Wt.ap[0], [128, 2], [1, 128]])

            def wv_rhs(j, m0, m):
                return bass.AP(
                    tensor=WvA.tensor,
                    offset=WvA.offset + W_J * j + m0,
                    ap=[WvA.ap[0], [512, 2], [1, m]])

            def qk_oct(ch, oct):
                # q/k projection for token chunk ch, one 128-channel out tile
                cols = slice(CW * ch, CW * (ch + 1))
                if True:
                    Wt = WqA if oct < 4 else WkA
                    o = oct % 4
                    bias = (bq if oct < 4 else bk)[:, o:o + 1]
                    ps = pqkv.tile([128, 512], F32, tag="qkps", name="qkps")
                    for sub in range(2):
                        for t3 in range(3):
                            for p2 in range(2):
                                j = 2 * t3 + p2
                                nc.tensor.matmul(
                                    out=ps[:, 256 * sub:256 * (sub + 1)],
                                    lhsT=wqk_lhs(Wt, o, j),
                                    rhs=x_rhs(2 * ch + sub, j, 0, 256),
                                    start=(t3 == 0 and p2 == 0),
                                    stop=(t3 == 2 and p2 == 1),
                                    perf_mode=DR)
                    if oct >= 4:
                        o_ = kT[o][:, cols]
                        if (oct + ch) % 2 == 0:
                            nc.vector.tensor_scalar_add(
                                out=o_, in0=ps[:, 0:CW], scalar1=bias)
                        else:
                            nc.scalar.activation(out=o_, in_=ps[:, 0:CW],
                                                 func=AF.Identity, bias=bias)
                    else:
                        # q scaled by 1/8 (score scale) during evacuation
                        if (oct + ch) % 2 == 0:
                            nc.scalar.activation(
                                out=qT[o][:, cols], in_=ps[:, 0:CW],
                                func=AF.Identity, bias=bias, scale=0.125)
                        else:
                            nc.vector.tensor_scalar(
                                out=qT[o][:, cols], in0=ps[:, 0:CW],
                                scalar1=0.125,
                                scalar2=bias,
                                op0=mybir.AluOpType.mult, op1=mybir.AluOpType.add)

            def v_half(bf, half, dst):
                if True:
                    ps = pqkv.tile([128, 512], F32, tag="qkps", name="vps")
                    w0 = 128 * half
                    for vc in range(2):
                        for t3 in range(3):
                            for p2 in range(2):
                                j = 2 * t3 + p2
                                nc.tensor.matmul(
                                    out=ps[:, 256 * vc:256 * (vc + 1)],
                                    lhsT=x_rhs(bf, j, w0, 128),
                                    rhs=wv_rhs(j, 256 * vc, 256),
                                    start=(t3 == 0 and p2 == 0),
                                    stop=(t3 == 2 and p2 == 1),
                                    perf_mode=DR)
                    if (bf + half) % 2 == 0:
                        nc.vector.tensor_copy(dst, ps)
                    else:
                        nc.scalar.activation(out=dst, in_=ps, func=AF.Copy)

            class Att:
                """Per-bf attention, emitted in stages so projection work can
                interleave between scores and AV (PE wait queue is 4 deep)."""

                def __init__(self, bf, vwin):
                    self.bf = bf
                    self.t0 = bf * T
                    self.vwin = vwin
                    self.oab = {}
                    self.den = None
                    self.exh = {}

                def scores(self, qb, pool=None):
                    t0 = self.t0
                    q0, nq, k0, W = QBLOCKS[qb]
                    fn = 4 * nq
                    exh = []
                    pl = pool if pool is not None else pst
                    tg = "qkps" if pool is not None else "st"
                    merged = (2 * fn <= 512)
                    if merged:
                        st1 = pl.tile([128, 512], F32, tag=tg, name="st")
                        sts = [st1, st1]
                        goff = fn
                    else:
                        sts = [pl.tile([128, 512], F32, tag=tg, name="st")
                               for _ in range(2)]
                        goff = 0
                    # 64-row quadrant tiles fault the PE on a direct switch to
                    # a different tile_position.  Every 64-row score matmul is
                    # guarded by a full-128-row 8-column matmul reading the
                    # SAME kT/qT tiles (identical readiness, adjacent priority,
                    # WAW-ordered via the overwritten corner), so the PE never
                    # sees two different 64-row quadrant configs back-to-back.
                    for g in range(2):
                        st = sts[g]
                        c0 = goff * g
                        for hh in range(4):
                            h = 4 * g + hh
                            p = h // 2
                            pr = 64 * (h % 2)
                            nc.tensor.matmul(
                                out=st[0:8, c0 + nq * hh:c0 + nq * hh + 8],
                                lhsT=kT[p][:, t0 + k0:t0 + k0 + 8],
                                rhs=qT[p][:, t0 + q0:t0 + q0 + 8],
                                start=True, stop=True)
                            nc.tensor.matmul(
                                out=st[0:W, c0 + nq * hh:c0 + nq * (hh + 1)],
                                lhsT=kT[p][pr:pr + 64, t0 + k0:t0 + k0 + W],
                                rhs=qT[p][pr:pr + 64, t0 + q0:t0 + q0 + nq],
                                start=True, stop=True,
                                tile_position=(pr, 0))
                    if merged:
                        et = epool.tile([128, 512], BF16, tag="et", name="et")
                        nc.scalar.activation(out=et[0:W, 0:2 * fn],
                                             in_=sts[0][0:W, 0:2 * fn],
                                             func=AF.Exp)
                        ex = epool.tile([128, 512], BF16, tag="ex", name="ex")
                        nc.vector.tensor_mul(
                            ex[0:W, 0:2 * fn], et[0:W, 0:2 * fn],
                            EB[0:W, QB_OFF[qb]:QB_OFF[qb] + 2 * fn])
                        exh = [ex[:, 0:fn], ex[:, fn:2 * fn]]
                    else:
                        for g in range(2):
                            st = sts[g]
                            et = epool.tile([128, 512], BF16, tag="et",
                                            name="et")
                            nc.scalar.activation(out=et[0:W, 0:fn],
                                                 in_=st[0:W, 0:fn], func=AF.Exp)
                            ex = epool.tile([128, 512], BF16, tag="ex",
                                            name="ex")
                            nc.vector.tensor_mul(
                                ex[0:W, 0:fn], et[0:W, 0:fn],
                                EB[0:W,
                                   QB_OFF[qb] + fn * g:QB_OFF[qb] + fn * g + fn])
                            exh.append(ex[:, 0:fn])
                    self.exh[qb] = exh

                def av(self, qb):
                    q0, nq, k0, W = QBLOCKS[qb]
                    exh = self.exh[qb]
                    if self.den is None:
                        self.den = pden.tile([128, 16], F32, tag="den",
                                             name="den")
                    for (pqb, cs, nqp, b0, yt) in PIECES:
                        if pqb != qb:
                            continue
                        if yt not in self.oab:
                            self.oab[yt] = poa.tile([128, 512], F32,
                                                    tag="oab", name="oab")
                        for g in range(2):
                            for hh in range(4):
                                h = 4 * g + hh
                                exs = exh[g][0:W,
                                             nq * hh + cs:nq * hh + cs + nqp]
                                nc.tensor.matmul(
                                    out=self.oab[yt][b0:b0 + nqp,
                                                     64 * h:64 * h + 64],
                                    lhsT=exs,
                                    rhs=self.vwin[qb][0:W, 64 * h:64 * h + 64],
                                    start=True, stop=True,
                                    tile_position=(0, b0))
                                nc.tensor.matmul(
                                    out=self.den[b0:b0 + nqp,
                                                 8 * yt + h:8 * yt + h + 1],
                                    lhsT=exs, rhs=onec[0:W, :],
                                    start=True, stop=True,
                                    tile_position=(0, b0))

                def finish(self, yt):
                    oa = self.oab[yt]
                    rc = rpool.tile([128, 8], F32, tag="rc", name="rc")
                    yf = ypool.tile([128, C], F16, tag="yf", name="yf")
                    nc.vector.reciprocal(rc, self.den[:, 8 * yt:8 * yt + 8])
                    in1 = bass.AP(tensor=rc.tensor, offset=rc.offset,
                                  ap=[rc.ap[0], [1, 8], [0, 64]])
                    nc.vector.scalar_tensor_tensor(
                        out=yf, in0=oa, scalar=1.0, in1=in1,
                        op0=mybir.AluOpType.mult, op1=mybir.AluOpType.mult)
                    nc.sync.dma_start(
                        out=y_ap[self.t0 + 128 * yt:self.t0 + 128 * (yt + 1), :],
                        in_=yf)

            # warm the ACT function table (Exp) during the DMA phase
            dummy = singles.tile([1, 1], F32, tag="dummy")
            nc.scalar.activation(out=dummy, in_=ones_r[0:1, 0:1], func=AF.Exp)

            # ---- software-pipelined emission ----
            # chunk 0 projection up front; afterwards the qk octs of chunk
            # ch feed attention bf=2ch..2ch+1 and are drip-fed as PE filler
            # between scores and AV stages (covering exp latency without
            # clogging the 4-deep PE wait queue).
            from collections import deque
            fillers = deque((ch, oct) for ch in range(1, NCH)
                            for oct in range(8))
            emitted = {0: 8}

            def run_filler(n):
                for _ in range(n):
                    if fillers:
                        ch, oct = fillers.popleft()
                        qk_oct(ch, oct)
                        emitted[ch] = emitted.get(ch, 0) + 1

            def drain_chunk(ch):
                while fillers and fillers[0][0] <= ch:
                    c, oct = fillers.popleft()
                    qk_oct(c, oct)
                    emitted[c] = emitted.get(c, 0) + 1

            for oct in (0, 4, 1, 5, 2, 6, 3, 7):
                qk_oct(0, oct)

            vtiles = {}

            def new_vset(bf):
                vA = vpool.tile([128, 512], BF16, tag="vA", name="vA")
                vC = vpool.tile([128, 512], BF16, tag="vC", name="vC")
                vB = vpool.tile([128, 512], BF16, tag="vB", name="vB")
                vD = vpool.tile([128, 512], BF16, tag="vD", name="vD")
                vtiles[bf] = (vA, vC, vB, vD)
                return vtiles[bf]

            def v_shuffles(bf):
                vA, vC, vB, vD = vtiles[bf]
                nc.sync.dma_start(out=vB[0:64, :], in_=vA[64:128, :])
                nc.sync.dma_start(out=vB[64:128, :], in_=vC[0:64, :])
                nc.sync.dma_start(out=vD[0:96, :], in_=vC[32:128, :])

            vA0, vC0, _, _ = new_vset(0)
            v_half(0, 0, vA0)
            v_half(0, 1, vC0)
            v_shuffles(0)

            for bf in range(BFL - 2):
                if bf % 2 == 0:
                    drain_chunk(bf // 2)   # this chunk's q/k must be ready
                vA, vC, vB, vD = vtiles[bf]
                att = Att(bf, {0: vA, 1: vB, 2: vD})
                att.scores(0)
                nvA, nvC, _, _ = new_vset(bf + 1)
                v_half(bf + 1, 0, nvA)
                att.av(0)
                att.scores(1)
                v_half(bf + 1, 1, vtiles[bf + 1][1])
                v_shuffles(bf + 1)
                att.av(1)
                att.finish(0)
                att.scores(2)
                run_filler(3)
                att.av(2)
                att.finish(1)
                del vtiles[bf]

            # hand-scheduled endgame: no projection filler remains, so the
            # two final bf interleave, with late stages drawn from the idle
            # qkv psum pool to fill exp-latency gaps
            drain_chunk(3)
            b6, b7 = BFL - 2, BFL - 1
            vA, vC, vB, vD = vtiles[b6]
            a6 = Att(b6, {0: vA, 1: vB, 2: vD})
            a6.scores(0)
            nvA, nvC, _, _ = new_vset(b7)
            v_half(b7, 0, nvA)
            a6.av(0)
            a6.scores(1)
            v_half(b7, 1, vtiles[b7][1])
            v_shuffles(b7)
            a6.scores(2, pool=pqkv)
            a6.av(1)
            a6.finish(0)
            vA, vC, vB, vD = vtiles[b7]
            a7 = Att(b7, {0: vA, 1: vB, 2: vD})
            a7.scores(0, pool=pqkv)
            a6.av(2)
            a6.finish(1)
            a7.scores(1)
            a7.av(0)
            a7.scores(2, pool=pqkv)
            a7.av(1)
            a7.finish(0)
            a7.av(2)
            a7.finish(1)

    nc.compile()
    return nc


def _f8rt(a):
    import ml_dtypes
    return np.asarray(a, np.float32).astype(
        ml_dtypes.float8_e4m3).astype(np.float32)


def _tri_terms_w(Wp):
    W1 = _f8rt(Wp)
    W2 = _f8rt(Wp / 4.0)
    W3 = _f8rt(16.0 * (Wp - W1))
    return np.stack([W1, W2, W3]).reshape(3, 2, 2, 128, 512)  # [t3,p2,kt,p,m]


def _tri_split_w(Wp):
    """Wp [512,512] f32 -> packed [128, 6*W_J] fp8 image (j, kt, m)."""
    import ml_dtypes
    Wt = _tri_terms_w(Wp).transpose(3, 0, 1, 2, 4)  # [p, t3, p2, kt, m]
    return np.ascontiguousarray(
        Wt.reshape(128, 6 * W_J).astype(ml_dtypes.float8_e4m3))


def _tri_split_w_m(Wp):
    """Wp [512,512] f32 -> m-major [128, 4*W_MB] fp8 image (mb, j, kt, 128)."""
    import ml_dtypes
    Wt = _tri_terms_w(Wp).reshape(3, 2, 2, 128, 4, 128)
    Wt = Wt.transpose(3, 4, 0, 1, 2, 5)             # [p, mb, t3, p2, kt, m]
    return np.ascontiguousarray(
        Wt.reshape(128, 4 * W_MB).astype(ml_dtypes.float8_e4m3))


def _prep_shared(pe, W_qkv, b_qkv):
    r = np.arange(512)
    head = 2 * (r // 128) + (r % 128) // 64
    cc = r % 64
    qsrc = 192 * head + cc
    ksrc = 192 * head + 64 + cc
    j = np.arange(512)
    vsrc = 192 * (j // 64) + 128 + (j % 64)

    import ml_dtypes
    # q weights quantize UNSCALED (0.125-prescaling would crush them into
    # fp8 subnormals); the 1/8 score scale rides the evacuation copy's
    # per-partition scale (maske/masko).
    WqAv = _tri_split_w_m(np.ascontiguousarray(W_qkv[:, qsrc]))
    WkAv = _tri_split_w_m(np.ascontiguousarray(W_qkv[:, ksrc]))
    WvAv = _tri_split_w(np.ascontiguousarray(W_qkv[:, vsrc]))
    bqv = (b_qkv[qsrc] * 0.125).astype(np.float32).reshape(4, 128).T.copy()
    bkv = b_qkv[ksrc].astype(np.float32).reshape(4, 128).T.copy()

    # BT columns: per qblock, head groups of 4 at 384*g, head-major inside.
    # -80 sentinel: exp(s - 80) underflows bf16 to ~0 for any realistic score.
    BAND_NEG = -80.0
    BTm = np.full((128, 2048), BAND_NEG, dtype=np.float32)
    for qb, (q0, nq, k0, W) in enumerate(QBLOCKS):
        for h in range(NH):
            jj = np.arange(W)[:, None]
            ii = np.arange(nq)[None, :]
            d = (q0 - k0) + ii - jj
            valid = (d >= 0) & (d <= LB)
            idx = np.clip(LB - d, 0, LB)
            blk = np.where(valid, pe[h][idx], BAND_NEG).astype(np.float32)
            off = QB_OFF[qb] + 4 * nq * (h // 4) + nq * (h % 4)
            BTm[0:W, off:off + nq] = blk
    EBm = np.exp(np.minimum(BTm, 60.0)).astype(ml_dtypes.bfloat16)
    bvh = b_qkv[vsrc].astype(np.float32)
    return dict(WqA=WqAv, WkA=WkAv, WvA=WvAv, bq=bqv,
                bk=bkv, EB=EBm,
                onec=np.ones((128, 1), dtype=ml_dtypes.bfloat16),
                ones=np.ones((1, 128), dtype=ml_dtypes.bfloat16)), bvh


def _pack_x(xs):
    """xs [2048, 512] f32 -> [128, 8*XF_S] fp8 sub-chunk-major image."""
    import ml_dtypes
    A = _f8rt(xs)
    Cc = _f8rt(4.0 * (xs - A))
    B = _f8rt(xs / 16.0)
    arr = np.stack([A, Cc, B])                     # [t3, tok, cin]
    arr = arr.reshape(3, 8, 256, 2, 2, 128)        # [t3, s, n, p2, kt, p]
    arr = arr.transpose(5, 1, 0, 3, 4, 2)          # [p, s, t3, p2, kt, n]
    return np.ascontiguousarray(
        arr.reshape(128, 8 * XF_S).astype(ml_dtypes.float8_e4m3))


def kernel(x, pe, W_qkv, b_qkv, num_heads):
    assert int(num_heads) == NH and x.shape == (BF, T, C)
    if "nc" not in _CACHE:
        _CACHE["nc"] = _build_module()
    nc = _CACHE["nc"]

    shared, bvh = _prep_shared(np.asarray(pe, np.float32),
                               np.asarray(W_qkv, np.float32),
                               np.asarray(b_qkv, np.float32))
    in_maps = []
    for c in range(NCORES):
        xs = np.asarray(x[BFL * c:BFL * (c + 1)], np.float32).reshape(TOK, C)
        m = dict(shared)
        m["xF"] = _pack_x(xs)
        in_maps.append(m)
    res = run_bass_kernel_spmd(nc, in_maps, list(range(NCORES)))
    out = np.stack([np.asarray(res.results[c]["y"], np.float32).reshape(BFL, T, C)
                    for c in range(NCORES)], axis=0)
    # v-bias: softmax weights sum to 1, so the bias is a constant per channel
    return (out.reshape(BF, T, C) + bvh[None, None, :]).astype(np.float32)


# revision 53
# speedup vs baseline: 1.0095x; 1.0095x over previous
"""Trainium2 Bass kernel for sliding-window causal attention (CausalAttention).

Computation (per reference):
    qkv = x @ W_qkv + b_qkv            # [BF, T, 3C]
    split into per-head q, k, v (head dim 64)
    scores = (q @ k.T) / sqrt(hd) + band_bias(pe)   # band: 0 <= i-j <= 31
    out = softmax(scores) @ v          # [BF, T, C]

Sharding: data-parallel over BF across 8 cores (8 bf rows per core).
Device strategy (per core, bf_l=8, tok=2048):
  - qkv projection in fp8e4m3 DoubleRow matmuls (3/4 the bf16 PE cycles,
    tighter than bf16 numerically) via the three-term split
    x@W = A@W1 + C@W2 + B@W3 with A=fp8(x), C=fp8(4(x-A)), B=fp8(x/16),
    W1=fp8(W), W2=fp8(W/4), W3=fp8(16(W-W1)).  All term planes are packed
    host-side into a sub-chunk-major fp8 image (xF) plus m-major q/k and
    j-major v weight images.  The 1/8 score scale rides the q evacuation
    (scaling weights before fp8 quantization would crush them into
    subnormals).
  - scores are computed transposed (ST[key, query]) per query-block window
    with 64-deep contractions using 64-row tile_position quadrants (even
    heads rows 0-63, odd heads 64-127), so q and k evacuate once each with
    no parity masks.  The PE faults on direct switches between different
    64-row quadrant configs, so every score matmul is preceded by a tiny
    full-128-row guard matmul that reads the same kT/qT tiles (identical
    readiness + WAW corner overlap keep the pair adjacent under the tile
    scheduler).
  - four heads share a PSUM bank per score block; exp runs on ACT, the
    exp(band-bias) multiply on DVE; the final qblock's 8 heads share a
    single bank/exp.
  - softmax denominators accumulate in a separate [128,16] PSUM tile via
    1-column ones-matmuls; the division is a reciprocal plus one
    scalar_tensor_tensor per output tile writing fp16 y (converted and
    v-bias-added on the host: softmax weights sum to 1, so the v bias is a
    per-channel constant).
  - emission is software-pipelined: projection octs and v-projections are
    drip-fed between score and AV stages (PE wait queue is 4 deep), and the
    final two bf interleave their stages out of the idle qkv PSUM pool.
"""

import os
import numpy as np
from contextlib import ExitStack

import concourse.bacc as bacc
import concourse.bass as bass
import concourse.tile as tile
import concourse.mybir as mybir
from concourse.bass_utils import run_bass_kernel_spmd

BF, T, C = 64, 256, 512
NH, LB, HD = 8, 31, 64
NCORES = 8
BFL = BF // NCORES            # 8 bf rows per core
TOK = BFL * T                 # 2048 tokens per core
NEG = -1.0e30

F32 = mybir.dt.float32
F32R = mybir.dt.float32r
F16 = mybir.dt.float16
BF16 = mybir.dt.bfloat16
F8 = mybir.dt.float8e4
AF = mybir.ActivationFunctionType
DR = mybir.MatmulPerfMode.DoubleRow

# query blocks: (q0, nq, k0, W)  -- window keys [k0, k0+W)
QBLOCKS = [
    (0, 96, 0, 96),
    (96, 96, 64, 128),
    (192, 64, 160, 96),
]
QB_OFF = [0, 768, 1536]       # column offset of each qblock in BT
# output pieces: (qb_idx, col_start_in_ex, nq_piece, psum_base==y_row, y_tile)
PIECES = [
    (0, 0, 96, 0, 0),
    (1, 0, 32, 96, 0),
    (1, 32, 64, 0, 1),
    (2, 0, 64, 64, 1),
]

# fp8 tri-term packing: per-partition strides inside xF / W images
#   xF layout  [128, s(8), j(6), kt(2), n(256)]  (j = term*2 + chpair)
#   Wq/Wk      [128, mb(4), j(6), kt(2), 128]    (m-major for startup DMAs)
#   Wv         [128, j(6), kt(2), m(512)]
XF_S = 6 * 2 * 256            # 3072
W_MB = 6 * 2 * 128            # 1536
W_J = 2 * 512                 # 1024

_CACHE = {}


def _build_module():
    nc = bacc.Bacc("TRN2", target_bir_lowering=False, debug=False,
                   num_devices=NCORES)

    dram = {}
    def din(name, shape, dt):
        dram[name] = nc.dram_tensor(name, shape, dt, kind="ExternalInput").ap()
    din("xF", [128, 8 * XF_S], F8)
    din("WqA", [128, 6 * W_J], F8)
    din("WkA", [128, 6 * W_J], F8)
    din("WvA", [128, 6 * W_J], F8)
    din("bq", [128, 4], F32)
    din("bk", [128, 4], F32)
    din("onec", [128, 1], BF16)
    din("EB", [128, 2048], BF16)
    din("ones", [1, 128], BF16)
    y_ap = nc.dram_tensor("y", [TOK, C], F16, kind="ExternalOutput").ap()

    NCH = max(1, TOK // 512)
    CW = TOK // NCH

    with tile.TileContext(nc) as tc:
        with ExitStack() as ctx:
            singles = ctx.enter_context(tc.tile_pool(name="singles", bufs=1))

            # ---- persistent SBUF tensors ----
            xF = singles.tile([128, 8 * XF_S], F8, tag="xF", name="xF")
            WqA = singles.tile([128, 6 * W_J], F8, tag="WqA", name="WqA")
            WkA = singles.tile([128, 6 * W_J], F8, tag="WkA", name="WkA")
            WvA = singles.tile([128, 6 * W_J], F8, tag="WvA", name="WvA")
            bq = singles.tile([128, 4], F32, tag="bq")
            bk = singles.tile([128, 4], F32, tag="bk")
            onec = singles.tile([128, 1], BF16, tag="onec")
            ones_r = singles.tile([1, 128], BF16, tag="ones_r")
            EB = singles.tile([128, 2048], BF16, tag="EB")
            qT = [singles.tile([128, TOK], BF16, tag=f"qT{i}", name=f"qT{i}")
                  for i in range(4)]
            kT = [singles.tile([128, TOK], BF16, tag=f"kT{i}", name=f"kT{i}")
                  for i in range(4)]

            def load_xf(sb):
                nc.sync.dma_start(
                    out=xF[:, XF_S * sb:XF_S * (sb + 1)],
                    in_=dram["xF"][:, XF_S * sb:XF_S * (sb + 1)])
            def load_wq(mb):
                nc.sync.dma_start(
                    out=WqA[:, W_MB * mb:W_MB * (mb + 1)],
                    in_=dram["WqA"][:, W_MB * mb:W_MB * (mb + 1)])
            def load_wk(mb):
                nc.sync.dma_start(
                    out=WkA[:, W_MB * mb:W_MB * (mb + 1)],
                    in_=dram["WkA"][:, W_MB * mb:W_MB * (mb + 1)])
            load_wq(0)
            load_xf(0)
            nc.sync.dma_start(out=bq, in_=dram["bq"])
            nc.sync.dma_start(out=bk, in_=dram["bk"])
            load_xf(1)
            load_wk(0)
            load_wq(1)
            load_wk(1)
            load_wq(2)
            load_wk(2)
            load_wq(3)
            load_wk(3)
            nc.sync.dma_start(out=WvA, in_=dram["WvA"])
            nc.sync.dma_start(out=onec, in_=dram["onec"])
            nc.sync.dma_start(out=EB[:, 0:768], in_=dram["EB"][:, 0:768])
            nc.sync.dma_start(out=ones_r, in_=dram["ones"])
            load_xf(2)
            load_xf(3)
            nc.sync.dma_start(out=EB[:, 768:2048], in_=dram["EB"][:, 768:2048])
            for sb in range(4, 8):
                load_xf(sb)
            pqkv = ctx.enter_context(
                tc.tile_pool(name="psum_qkv", bufs=3, space="PSUM"))
            pst = ctx.enter_context(
                tc.tile_pool(name="psum_st", bufs=2, space="PSUM"))
            poa = ctx.enter_context(
                tc.tile_pool(name="psum_oa", bufs=2, space="PSUM"))
            pden = ctx.enter_context(
                tc.tile_pool(name="psum_den", bufs=1, space="PSUM"))
            vpool = ctx.enter_context(tc.tile_pool(name="vsb", bufs=10))
            epool = ctx.enter_context(tc.tile_pool(name="esb", bufs=16))
            rpool = ctx.enter_context(tc.tile_pool(name="rsb", bufs=8))
            ypool = ctx.enter_context(tc.tile_pool(name="ysb", bufs=8))

            def x_rhs(sb, j, w0, n):
                # xF[:, sb, j, :, w0:w0+n]  (kt-major pair for DoubleRow)
                return bass.AP(
                    tensor=xF.tensor,
                    offset=xF.offset + XF_S * sb + 512 * j + w0,
                    ap=[xF.ap[0], [256, 2], [1, n]])

            def wqk_lhs(Wt, o, j):
                # Wq/Wk[:, o, j, :, :]  (m-major image, 128-wide out tile)
                return bass.AP(
                    tensor=Wt.tensor,
                    offset=Wt.offset + W_MB * o + 256 * j,
                    ap=[Wt.ap[0], [128, 2], [1, 128]])

            def wv_rhs(j, m0, m):
                return bass.AP(
                    tensor=WvA.tensor,
                    offset=WvA.offset + W_J * j + m0,
                    ap=[WvA.ap[0], [512, 2], [1, m]])

            def qk_oct(ch, oct):
                # q/k projection for token chunk ch, one 128-channel out tile
                cols = slice(CW * ch, CW * (ch + 1))
                if True:
                    Wt = WqA if oct < 4 else WkA
                    o = oct % 4
                    bias = (bq if oct < 4 else bk)[:, o:o + 1]
                    ps = pqkv.tile([128, 512], F32, tag="qkps", name="qkps")
                    for sub in range(2):
                        for t3 in range(3):
                            for p2 in range(2):
                                j = 2 * t3 + p2
                                nc.tensor.matmul(
                                    out=ps[:, 256 * sub:256 * (sub + 1)],
                                    lhsT=wqk_lhs(Wt, o, j),
                                    rhs=x_rhs(2 * ch + sub, j, 0, 256),
                                    start=(t3 == 0 and p2 == 0),
                                    stop=(t3 == 2 and p2 == 1),
                                    perf_mode=DR)
                    if oct >= 4:
                        o_ = kT[o][:, cols]
                        if (oct + ch) % 2 == 0:
                            nc.vector.tensor_scalar_add(
                                out=o_, in0=ps[:, 0:CW], scalar1=bias)
                        else:
                            nc.scalar.activation(out=o_, in_=ps[:, 0:CW],
                                                 func=AF.Identity, bias=bias)
                    else:
                        # q scaled by 1/8 (score scale) during evacuation
                        if (oct + ch) % 2 == 0:
                            nc.scalar.activation(
                                out=qT[o][:, cols], in_=ps[:, 0:CW],
                                func=AF.Identity, bias=bias, scale=0.125)
                        else:
                            nc.vector.tensor_scalar(
                                out=qT[o][:, cols], in0=ps[:, 0:CW],
                                scalar1=0.125,
                                scalar2=bias,
                                op0=mybir.AluOpType.mult, op1=mybir.AluOpType.add)

            def v_half(bf, half, dst):
                if True:
                    ps = pqkv.tile([128, 512], F32, tag="qkps", name="vps")
                    w0 = 128 * half
                    for vc in range(2):
                        for t3 in range(3):
                            for p2 in range(2):
                                j = 2 * t3 + p2
                                nc.tensor.matmul(
                                    out=ps[:, 256 * vc:256 * (vc + 1)],
                                    lhsT=x_rhs(bf, j, w0, 128),
                                    rhs=wv_rhs(j, 256 * vc, 256),
                                    start=(t3 == 0 and p2 == 0),
                                    stop=(t3 == 2 and p2 == 1),
                                    perf_mode=DR)
                    if (bf + half) % 2 == 0:
                        nc.vector.tensor_copy(dst, ps)
                    else:
                        nc.scalar.activation(out=dst, in_=ps, func=AF.Copy)

            class Att:
                """Per-bf attention, emitted in stages so projection work can
                interleave between scores and AV (PE wait queue is 4 deep)."""

                def __init__(self, bf, vwin):
                    self.bf = bf
                    self.t0 = bf * T
                    self.vwin = vwin
                    self.oab = {}
                    self.den = None
                    self.exh = {}

                def scores(self, qb, pool=None):
                    t0 = self.t0
                    q0, nq, k0, W = QBLOCKS[qb]
                    fn = 4 * nq
                    exh = []
                    pl = pool if pool is not None else pst
                    tg = "qkps" if pool is not None else "st"
                    merged = (2 * fn <= 512)
                    if merged:
                        st1 = pl.tile([128, 512], F32, tag=tg, name="st")
                        sts = [st1, st1]
                        goff = fn
                    else:
                        sts = [pl.tile([128, 512], F32, tag=tg, name="st")
                               for _ in range(2)]
                        goff = 0
                    # 64-row quadrant tiles fault the PE on a direct switch to
                    # a different tile_position.  Every 64-row score matmul is
                    # guarded by a full-128-row 8-column matmul reading the
                    # SAME kT/qT tiles (identical readiness, adjacent priority,
                    # WAW-ordered via the overwritten corner), so the PE never
                    # sees two different 64-row quadrant configs back-to-back.
                    for g in range(2):
                        st = sts[g]
                        c0 = goff * g
                        for hh in range(4):
                            h = 4 * g + hh
                            p = h // 2
                            pr = 64 * (h % 2)
                            nc.tensor.matmul(
                                out=st[0:8, c0 + nq * hh:c0 + nq * hh + 8],
                                lhsT=kT[p][:, t0 + k0:t0 + k0 + 8],
                                rhs=qT[p][:, t0 + q0:t0 + q0 + 8],
                                start=True, stop=True)
                            nc.tensor.matmul(
                                out=st[0:W, c0 + nq * hh:c0 + nq * (hh + 1)],
                                lhsT=kT[p][pr:pr + 64, t0 + k0:t0 + k0 + W],
                                rhs=qT[p][pr:pr + 64, t0 + q0:t0 + q0 + nq],
                                start=True, stop=True,
                                tile_position=(pr, 0))
                    if merged:
                        et = epool.tile([128, 512], BF16, tag="et", name="et")
                        nc.scalar.activation(out=et[0:W, 0:2 * fn],
                                             in_=sts[0][0:W, 0:2 * fn],
                                             func=AF.Exp)
                        ex = epool.tile([128, 512], BF16, tag="ex", name="ex")
                        nc.vector.tensor_mul(
                            ex[0:W, 0:2 * fn], et[0:W, 0:2 * fn],
                            EB[0:W, QB_OFF[qb]:QB_OFF[qb] + 2 * fn])
                        exh = [ex[:, 0:fn], ex[:, fn:2 * fn]]
                    else:
                        for g in range(2):
                            st = sts[g]
                            et = epool.tile([128, 512], BF16, tag="et",
                                            name="et")
                            nc.scalar.activation(out=et[0:W, 0:fn],
                                                 in_=st[0:W, 0:fn], func=AF.Exp)
                            ex = epool.tile([128, 512], BF16, tag="ex",
                                            name="ex")
                            nc.vector.tensor_mul(
                                ex[0:W, 0:fn], et[0:W, 0:fn],
                                EB[0:W,
                                   QB_OFF[qb] + fn * g:QB_OFF[qb] + fn * g + fn])
                            exh.append(ex[:, 0:fn])
                    self.exh[qb] = exh

                def av(self, qb):
                    q0, nq, k0, W = QBLOCKS[qb]
                    exh = self.exh[qb]
                    if self.den is None:
                        self.den = pden.tile([128, 16], F32, tag="den",
                                             name="den")
                    for (pqb, cs, nqp, b0, yt) in PIECES:
                        if pqb != qb:
                            continue
                        if yt not in self.oab:
                            self.oab[yt] = poa.tile([128, 512], F32,
                                                    tag="oab", name="oab")
                        for g in range(2):
                            for hh in range(4):
                                h = 4 * g + hh
                                exs = exh[g][0:W,
                                             nq * hh + cs:nq * hh + cs + nqp]
                                nc.tensor.matmul(
                                    out=self.oab[yt][b0:b0 + nqp,
                                                     64 * h:64 * h + 64],
                                    lhsT=exs,
                                    rhs=self.vwin[qb][0:W, 64 * h:64 * h + 64],
                                    start=True, stop=True,
                                    tile_position=(0, b0))
                                nc.tensor.matmul(
                                    out=self.den[b0:b0 + nqp,
                                                 8 * yt + h:8 * yt + h + 1],
                                    lhsT=exs, rhs=onec[0:W, :],
                                    start=True, stop=True,
                                    tile_position=(0, b0))

                def finish(self, yt):
                    oa = self.oab[yt]
                    rc = rpool.tile([128, 8], F32, tag="rc", name="rc")
                    yf = ypool.tile([128, C], F16, tag="yf", name="yf")
                    nc.vector.reciprocal(rc, self.den[:, 8 * yt:8 * yt + 8])
                    in1 = bass.AP(tensor=rc.tensor, offset=rc.offset,
                                  ap=[rc.ap[0], [1, 8], [0, 64]])
                    nc.vector.scalar_tensor_tensor(
                        out=yf, in0=oa, scalar=1.0, in1=in1,
                        op0=mybir.AluOpType.mult, op1=mybir.AluOpType.mult)
                    nc.sync.dma_start(
                        out=y_ap[self.t0 + 128 * yt:self.t0 + 128 * (yt + 1), :],
                        in_=yf)

            # warm the ACT function table (Exp) during the DMA phase
            dummy = singles.tile([1, 1], F32, tag="dummy")
            nc.scalar.activation(out=dummy, in_=ones_r[0:1, 0:1], func=AF.Exp)

            # ---- software-pipelined emission ----
            # chunk 0 projection up front; afterwards the qk octs of chunk
            # ch feed attention bf=2ch..2ch+1 and are drip-fed as PE filler
            # between scores and AV stages (covering exp latency without
            # clogging the 4-deep PE wait queue).
            from collections import deque
            fillers = deque((ch, oct) for ch in range(1, NCH)
                            for oct in range(8))
            emitted = {0: 8}

            def run_filler(n):
                for _ in range(n):
                    if fillers:
                        ch, oct = fillers.popleft()
                        qk_oct(ch, oct)
                        emitted[ch] = emitted.get(ch, 0) + 1

            def drain_chunk(ch):
                while fillers and fillers[0][0] <= ch:
                    c, oct = fillers.popleft()
                    qk_oct(c, oct)
                    emitted[c] = emitted.get(c, 0) + 1

            for oct in (0, 4, 1, 5, 2, 6, 3, 7):
                qk_oct(0, oct)

            vtiles = {}

            def new_vset(bf):
                vA = vpool.tile([128, 512], BF16, tag="vA", name="vA")
                vC = vpool.tile([128, 512], BF16, tag="vC", name="vC")
                vB = vpool.tile([128, 512], BF16, tag="vB", name="vB")
                vD = vpool.tile([128, 512], BF16, tag="vD", name="vD")
                vtiles[bf] = (vA, vC, vB, vD)
                return vtiles[bf]

            def v_shuffles(bf):
                vA, vC, vB, vD = vtiles[bf]
                nc.sync.dma_start(out=vB[0:64, :], in_=vA[64:128, :])
                nc.sync.dma_start(out=vB[64:128, :], in_=vC[0:64, :])
                nc.sync.dma_start(out=vD[0:96, :], in_=vC[32:128, :])

            vA0, vC0, _, _ = new_vset(0)
            v_half(0, 0, vA0)
            v_half(0, 1, vC0)
            v_shuffles(0)

            for bf in range(BFL - 2):
                if bf % 2 == 0:
                    drain_chunk(bf // 2)   # this chunk's q/k must be ready
                vA, vC, vB, vD = vtiles[bf]
                att = Att(bf, {0: vA, 1: vB, 2: vD})
                att.scores(0)
                nvA, nvC, _, _ = new_vset(bf + 1)
                v_half(bf + 1, 0, nvA)
                att.av(0)
                run_filler(1)
                att.scores(1)
                v_half(bf + 1, 1, vtiles[bf + 1][1])
                v_shuffles(bf + 1)
                att.av(1)
                run_filler(1)
                att.finish(0)
                att.scores(2)
                run_filler(1)
                att.av(2)
                att.finish(1)
                del vtiles[bf]

            # hand-scheduled endgame: no projection filler remains, so the
            # two final bf interleave, with late stages drawn from the idle
            # qkv psum pool to fill exp-latency gaps
            drain_chunk(3)
            b6, b7 = BFL - 2, BFL - 1
            vA, vC, vB, vD = vtiles[b6]
            a6 = Att(b6, {0: vA, 1: vB, 2: vD})
            a6.scores(0)
            nvA, nvC, _, _ = new_vset(b7)
            v_half(b7, 0, nvA)
            a6.av(0)
            a6.scores(1)
            v_half(b7, 1, vtiles[b7][1])
            v_shuffles(b7)
            a6.scores(2, pool=pqkv)
            a6.av(1)
            a6.finish(0)
            vA, vC, vB, vD = vtiles[b7]
            a7 = Att(b7, {0: vA, 1: vB, 2: vD})
            a7.scores(0, pool=pqkv)
            a6.av(2)
            a6.finish(1)
            a7.scores(1)
            a7.av(0)
            a7.scores(2, pool=pqkv)
            a7.av(1)
            a7.finish(0)
            a7.av(2)
            a7.finish(1)

    nc.compile()
    return nc


def _f8rt(a):
    import ml_dtypes
    return np.asarray(a, np.float32).astype(
        ml_dtypes.float8_e4m3).astype(np.float32)


def _tri_terms_w(Wp):
    W1 = _f8rt(Wp)
    W2 = _f8rt(Wp / 4.0)
    W3 = _f8rt(16.0 * (Wp - W1))
    return np.stack([W1, W2, W3]).reshape(3, 2, 2, 128, 512)  # [t3,p2,kt,p,m]


def _tri_split_w(Wp):
    """Wp [512,512] f32 -> packed [128, 6*W_J] fp8 image (j, kt, m)."""
    import ml_dtypes
    Wt = _tri_terms_w(Wp).transpose(3, 0, 1, 2, 4)  # [p, t3, p2, kt, m]
    return np.ascontiguousarray(
        Wt.reshape(128, 6 * W_J).astype(ml_dtypes.float8_e4m3))


def _tri_split_w_m(Wp):
    """Wp [512,512] f32 -> m-major [128, 4*W_MB] fp8 image (mb, j, kt, 128)."""
    import ml_dtypes
    Wt = _tri_terms_w(Wp).reshape(3, 2, 2, 128, 4, 128)
    Wt = Wt.transpose(3, 4, 0, 1, 2, 5)             # [p, mb, t3, p2, kt, m]
    return np.ascontiguousarray(
        Wt.reshape(128, 4 * W_MB).astype(ml_dtypes.float8_e4m3))


def _prep_shared(pe, W_qkv, b_qkv):
    r = np.arange(512)
    head = 2 * (r // 128) + (r % 128) // 64
    cc = r % 64
    qsrc = 192 * head + cc
    ksrc = 192 * head + 64 + cc
    j = np.arange(512)
    vsrc = 192 * (j // 64) + 128 + (j % 64)

    import ml_dtypes
    # q weights quantize UNSCALED (0.125-prescaling would crush them into
    # fp8 subnormals); the 1/8 score scale rides the evacuation copy's
    # per-partition scale (maske/masko).
    WqAv = _tri_split_w_m(np.ascontiguousarray(W_qkv[:, qsrc]))
    WkAv = _tri_split_w_m(np.ascontiguousarray(W_qkv[:, ksrc]))
    WvAv = _tri_split_w(np.ascontiguousarray(W_qkv[:, vsrc]))
    bqv = (b_qkv[qsrc] * 0.125).astype(np.float32).reshape(4, 128).T.copy()
    bkv = b_qkv[ksrc].astype(np.float32).reshape(4, 128).T.copy()

    # BT columns: per qblock, head groups of 4 at 384*g, head-major inside.
    # -80 sentinel: exp(s - 80) underflows bf16 to ~0 for any realistic score.
    BAND_NEG = -80.0
    BTm = np.full((128, 2048), BAND_NEG, dtype=np.float32)
    for qb, (q0, nq, k0, W) in enumerate(QBLOCKS):
        for h in range(NH):
            jj = np.arange(W)[:, None]
            ii = np.arange(nq)[None, :]
            d = (q0 - k0) + ii - jj
            valid = (d >= 0) & (d <= LB)
            idx = np.clip(LB - d, 0, LB)
            blk = np.where(valid, pe[h][idx], BAND_NEG).astype(np.float32)
            off = QB_OFF[qb] + 4 * nq * (h // 4) + nq * (h % 4)
            BTm[0:W, off:off + nq] = blk
    EBm = np.exp(np.minimum(BTm, 60.0)).astype(ml_dtypes.bfloat16)
    bvh = b_qkv[vsrc].astype(np.float32)
    return dict(WqA=WqAv, WkA=WkAv, WvA=WvAv, bq=bqv,
                bk=bkv, EB=EBm,
                onec=np.ones((128, 1), dtype=ml_dtypes.bfloat16),
                ones=np.ones((1, 128), dtype=ml_dtypes.bfloat16)), bvh


def _pack_x(xs):
    """xs [2048, 512] f32 -> [128, 8*XF_S] fp8 sub-chunk-major image."""
    import ml_dtypes
    A = _f8rt(xs)
    Cc = _f8rt(4.0 * (xs - A))
    B = _f8rt(xs / 16.0)
    arr = np.stack([A, Cc, B])                     # [t3, tok, cin]
    arr = arr.reshape(3, 8, 256, 2, 2, 128)        # [t3, s, n, p2, kt, p]
    arr = arr.transpose(5, 1, 0, 3, 4, 2)          # [p, s, t3, p2, kt, n]
    return np.ascontiguousarray(
        arr.reshape(128, 8 * XF_S).astype(ml_dtypes.float8_e4m3))


def kernel(x, pe, W_qkv, b_qkv, num_heads):
    assert int(num_heads) == NH and x.shape == (BF, T, C)
    if "nc" not in _CACHE:
        _CACHE["nc"] = _build_module()
    nc = _CACHE["nc"]

    shared, bvh = _prep_shared(np.asarray(pe, np.float32),
                               np.asarray(W_qkv, np.float32),
                               np.asarray(b_qkv, np.float32))
    in_maps = []
    for c in range(NCORES):
        xs = np.asarray(x[BFL * c:BFL * (c + 1)], np.float32).reshape(TOK, C)
        m = dict(shared)
        m["xF"] = _pack_x(xs)
        in_maps.append(m)
    res = run_bass_kernel_spmd(nc, in_maps, list(range(NCORES)))
    out = np.stack([np.asarray(res.results[c]["y"], np.float32).reshape(BFL, T, C)
                    for c in range(NCORES)], axis=0)
    # v-bias: softmax weights sum to 1, so the bias is a constant per channel
    return (out.reshape(BF, T, C) + bvh[None, None, :]).astype(np.float32)


# revision 59
# speedup vs baseline: 1.0130x; 1.0035x over previous
"""Trainium2 Bass kernel for sliding-window causal attention (CausalAttention).

Computation (per reference):
    qkv = x @ W_qkv + b_qkv            # [BF, T, 3C]
    split into per-head q, k, v (head dim 64)
    scores = (q @ k.T) / sqrt(hd) + band_bias(pe)   # band: 0 <= i-j <= 31
    out = softmax(scores) @ v          # [BF, T, C]

Sharding: data-parallel over BF across 8 cores (8 bf rows per core).
Device strategy (per core, bf_l=8, tok=2048):
  - qkv projection in fp8e4m3 DoubleRow matmuls (3/4 the bf16 PE cycles,
    tighter than bf16 numerically) via the three-term split
    x@W = A@W1 + C@W2 + B@W3 with A=fp8(x), C=fp8(4(x-A)), B=fp8(x/16),
    W1=fp8(W), W2=fp8(W/4), W3=fp8(16(W-W1)).  All term planes are packed
    host-side into a sub-chunk-major fp8 image (xF) plus m-major q/k and
    j-major v weight images.  The 1/8 score scale rides the q evacuation
    (scaling weights before fp8 quantization would crush them into
    subnormals).
  - scores are computed transposed (ST[key, query]) per query-block window
    with 64-deep contractions using 64-row tile_position quadrants (even
    heads rows 0-63, odd heads 64-127), so q and k evacuate once each with
    no parity masks.  The PE faults on direct switches between different
    64-row quadrant configs, so every score matmul is preceded by a tiny
    full-128-row guard matmul that reads the same kT/qT tiles (identical
    readiness + WAW corner overlap keep the pair adjacent under the tile
    scheduler).
  - four heads share a PSUM bank per score block; exp runs on ACT, the
    exp(band-bias) multiply on DVE; the final qblock's 8 heads share a
    single bank/exp.
  - softmax denominators accumulate in a separate [128,16] PSUM tile via
    1-column ones-matmuls; the division is a reciprocal plus one
    scalar_tensor_tensor per output tile writing fp16 y (converted and
    v-bias-added on the host: softmax weights sum to 1, so the v bias is a
    per-channel constant).
  - emission is software-pipelined: projection octs and v-projections are
    drip-fed between score and AV stages (PE wait queue is 4 deep), and the
    final two bf interleave their stages out of the idle qkv PSUM pool.
"""

import os
import numpy as np
from contextlib import ExitStack

import concourse.bacc as bacc
import concourse.bass as bass
import concourse.tile as tile
import concourse.mybir as mybir
from concourse.bass_utils import run_bass_kernel_spmd

BF, T, C = 64, 256, 512
NH, LB, HD = 8, 31, 64
NCORES = 8
BFL = BF // NCORES            # 8 bf rows per core
TOK = BFL * T                 # 2048 tokens per core
NEG = -1.0e30

F32 = mybir.dt.float32
F32R = mybir.dt.float32r
F16 = mybir.dt.float16
BF16 = mybir.dt.bfloat16
F8 = mybir.dt.float8e4
AF = mybir.ActivationFunctionType
DR = mybir.MatmulPerfMode.DoubleRow

# query blocks: (q0, nq, k0, W)  -- window keys [k0, k0+W)
QBLOCKS = [
    (0, 96, 0, 96),
    (96, 96, 64, 128),
    (192, 64, 160, 96),
]
QB_OFF = [0, 768, 1536]       # column offset of each qblock in BT
# output pieces: (qb_idx, col_start_in_ex, nq_piece, psum_base==y_row, y_tile)
PIECES = [
    (0, 0, 96, 0, 0),
    (1, 0, 32, 96, 0),
    (1, 32, 64, 0, 1),
    (2, 0, 64, 64, 1),
]

# fp8 tri-term packing: per-partition strides inside xF / W images
#   xF layout  [128, s(8), j(6), kt(2), n(256)]  (j = term*2 + chpair)
#   Wq/Wk      [128, mb(4), j(6), kt(2), 128]    (m-major for startup DMAs)
#   Wv         [128, j(6), kt(2), m(512)]
XF_S = 6 * 2 * 256            # 3072
W_MB = 6 * 2 * 128            # 1536
W_J = 2 * 512                 # 1024

_CACHE = {}


def _build_module():
    nc = bacc.Bacc("TRN2", target_bir_lowering=False, debug=False,
                   num_devices=NCORES)

    dram = {}
    def din(name, shape, dt):
        dram[name] = nc.dram_tensor(name, shape, dt, kind="ExternalInput").ap()
    din("xF", [128, 8 * XF_S], F8)
    din("WqA", [128, 6 * W_J], F8)
    din("WkA", [128, 6 * W_J], F8)
    din("WvA", [128, 6 * W_J], F8)
    din("bq", [128, 4], F32)
    din("bk", [128, 4], F32)
    din("onec", [128, 1], BF16)
    din("EB", [128, 2048], BF16)
    din("ones", [1, 128], BF16)
    y_ap = nc.dram_tensor("y", [TOK, C], F16, kind="ExternalOutput").ap()

    NCH = max(1, TOK // 512)
    CW = TOK // NCH

    with tile.TileContext(nc) as tc:
        with ExitStack() as ctx:
            singles = ctx.enter_context(tc.tile_pool(name="singles", bufs=1))

            # ---- persistent SBUF tensors ----
            xF = singles.tile([128, 8 * XF_S], F8, tag="xF", name="xF")
            WqA = singles.tile([128, 6 * W_J], F8, tag="WqA", name="WqA")
            WkA = singles.tile([128, 6 * W_J], F8, tag="WkA", name="WkA")
            WvA = singles.tile([128, 6 * W_J], F8, tag="WvA", name="WvA")
            bq = singles.tile([128, 4], F32, tag="bq")
            bk = singles.tile([128, 4], F32, tag="bk")
            onec = singles.tile([128, 1], BF16, tag="onec")
            ones_r = singles.tile([1, 128], BF16, tag="ones_r")
            EB = singles.tile([128, 2048], BF16, tag="EB")
            qT = [singles.tile([128, TOK], BF16, tag=f"qT{i}", name=f"qT{i}")
                  for i in range(4)]
            kT = [singles.tile([128, TOK], BF16, tag=f"kT{i}", name=f"kT{i}")
                  for i in range(4)]

            def load_xf(sb):
                nc.sync.dma_start(
                    out=xF[:, XF_S * sb:XF_S * (sb + 1)],
                    in_=dram["xF"][:, XF_S * sb:XF_S * (sb + 1)])
            def load_wq(mb):
                nc.sync.dma_start(
                    out=WqA[:, W_MB * mb:W_MB * (mb + 1)],
                    in_=dram["WqA"][:, W_MB * mb:W_MB * (mb + 1)])
            def load_wk(mb):
                nc.sync.dma_start(
                    out=WkA[:, W_MB * mb:W_MB * (mb + 1)],
                    in_=dram["WkA"][:, W_MB * mb:W_MB * (mb + 1)])
            load_wq(0)
            load_xf(0)
            nc.sync.dma_start(out=bq, in_=dram["bq"])
            nc.sync.dma_start(out=bk, in_=dram["bk"])
            load_xf(1)
            load_wk(0)
            load_wq(1)
            load_wk(1)
            load_wq(2)
            load_wk(2)
            load_wq(3)
            load_wk(3)
            nc.sync.dma_start(out=WvA, in_=dram["WvA"])
            nc.sync.dma_start(out=onec, in_=dram["onec"])
            nc.sync.dma_start(out=EB[:, 0:768], in_=dram["EB"][:, 0:768])
            nc.sync.dma_start(out=ones_r, in_=dram["ones"])
            load_xf(2)
            load_xf(3)
            nc.sync.dma_start(out=EB[:, 768:2048], in_=dram["EB"][:, 768:2048])
            for sb in range(4, 8):
                load_xf(sb)
            pqkv = ctx.enter_context(
                tc.tile_pool(name="psum_qkv", bufs=3, space="PSUM"))
            pst = ctx.enter_context(
                tc.tile_pool(name="psum_st", bufs=2, space="PSUM"))
            poa = ctx.enter_context(
                tc.tile_pool(name="psum_oa", bufs=2, space="PSUM"))
            pden = ctx.enter_context(
                tc.tile_pool(name="psum_den", bufs=1, space="PSUM"))
            vpool = ctx.enter_context(tc.tile_pool(name="vsb", bufs=10))
            epool = ctx.enter_context(tc.tile_pool(name="esb", bufs=16))
            rpool = ctx.enter_context(tc.tile_pool(name="rsb", bufs=8))
            ypool = ctx.enter_context(tc.tile_pool(name="ysb", bufs=8))

            def x_rhs(sb, j, w0, n):
                # xF[:, sb, j, :, w0:w0+n]  (kt-major pair for DoubleRow)
                return bass.AP(
                    tensor=xF.tensor,
                    offset=xF.offset + XF_S * sb + 512 * j + w0,
                    ap=[xF.ap[0], [256, 2], [1, n]])

            def wqk_lhs(Wt, o, j):
                # Wq/Wk[:, o, j, :, :]  (m-major image, 128-wide out tile)
                return bass.AP(
                    tensor=Wt.tensor,
                    offset=Wt.offset + W_MB * o + 256 * j,
                    ap=[Wt.ap[0], [128, 2], [1, 128]])

            def wv_rhs(j, m0, m):
                return bass.AP(
                    tensor=WvA.tensor,
                    offset=WvA.offset + W_J * j + m0,
                    ap=[WvA.ap[0], [512, 2], [1, m]])

            def qk_oct(ch, oct):
                # q/k projection for token chunk ch, one 128-channel out tile
                cols = slice(CW * ch, CW * (ch + 1))
                if True:
                    Wt = WqA if oct < 4 else WkA
                    o = oct % 4
                    bias = (bq if oct < 4 else bk)[:, o:o + 1]
                    ps = pqkv.tile([128, 512], F32, tag="qkps", name="qkps")
                    for sub in range(2):
                        for t3 in range(3):
                            for p2 in range(2):
                                j = 2 * t3 + p2
                                nc.tensor.matmul(
                                    out=ps[:, 256 * sub:256 * (sub + 1)],
                                    lhsT=wqk_lhs(Wt, o, j),
                                    rhs=x_rhs(2 * ch + sub, j, 0, 256),
                                    start=(t3 == 0 and p2 == 0),
                                    stop=(t3 == 2 and p2 == 1),
                                    perf_mode=DR)
                    if oct >= 4:
                        o_ = kT[o][:, cols]
                        if (oct + ch) % 2 == 0:
                            nc.vector.tensor_scalar_add(
                                out=o_, in0=ps[:, 0:CW], scalar1=bias)
                        else:
                            nc.scalar.activation(out=o_, in_=ps[:, 0:CW],
                                                 func=AF.Identity, bias=bias)
                    else:
                        # q scaled by 1/8 (score scale) during evacuation
                        if (oct + ch) % 2 == 0:
                            nc.scalar.activation(
                                out=qT[o][:, cols], in_=ps[:, 0:CW],
                                func=AF.Identity, bias=bias, scale=0.125)
                        else:
                            nc.vector.tensor_scalar(
                                out=qT[o][:, cols], in0=ps[:, 0:CW],
                                scalar1=0.125,
                                scalar2=bias,
                                op0=mybir.AluOpType.mult, op1=mybir.AluOpType.add)

            def v_half(bf, half, dst):
                if True:
                    ps = pqkv.tile([128, 512], F32, tag="qkps", name="vps")
                    w0 = 128 * half
                    for vc in range(2):
                        for t3 in range(3):
                            for p2 in range(2):
                                j = 2 * t3 + p2
                                nc.tensor.matmul(
                                    out=ps[:, 256 * vc:256 * (vc + 1)],
                                    lhsT=x_rhs(bf, j, w0, 128),
                                    rhs=wv_rhs(j, 256 * vc, 256),
                                    start=(t3 == 0 and p2 == 0),
                                    stop=(t3 == 2 and p2 == 1),
                                    perf_mode=DR)
                    if (bf + half) % 2 == 0:
                        nc.vector.tensor_copy(dst, ps)
                    else:
                        nc.scalar.activation(out=dst, in_=ps, func=AF.Copy)

            class Att:
                """Per-bf attention, emitted in stages so projection work can
                interleave between scores and AV (PE wait queue is 4 deep)."""

                def __init__(self, bf, vwin):
                    self.bf = bf
                    self.t0 = bf * T
                    self.vwin = vwin
                    self.oab = {}
                    self.den = None
                    self.exh = {}

                def scores(self, qb, pool=None):
                    t0 = self.t0
                    q0, nq, k0, W = QBLOCKS[qb]
                    fn = 4 * nq
                    exh = []
                    pl = pool if pool is not None else pst
                    tg = "qkps" if pool is not None else "st"
                    merged = (2 * fn <= 512)
                    if merged:
                        st1 = pl.tile([128, 512], F32, tag=tg, name="st")
                        sts = [st1, st1]
                        goff = fn
                    else:
                        sts = [pl.tile([128, 512], F32, tag=tg, name="st")
                               for _ in range(2)]
                        goff = 0
                    # 64-row quadrant tiles fault the PE on a direct switch to
                    # a different tile_position.  Every 64-row score matmul is
                    # guarded by a full-128-row 8-column matmul reading the
                    # SAME kT/qT tiles (identical readiness, adjacent priority,
                    # WAW-ordered via the overwritten corner), so the PE never
                    # sees two different 64-row quadrant configs back-to-back.
                    for g in range(2):
                        st = sts[g]
                        c0 = goff * g
                        for hh in range(4):
                            h = 4 * g + hh
                            p = h // 2
                            pr = 64 * (h % 2)
                            nc.tensor.matmul(
                                out=st[0:8, c0 + nq * hh:c0 + nq * hh + 8],
                                lhsT=kT[p][:, t0 + k0:t0 + k0 + 8],
                                rhs=qT[p][:, t0 + q0:t0 + q0 + 8],
                                start=True, stop=True)
                            nc.tensor.matmul(
                                out=st[0:W, c0 + nq * hh:c0 + nq * (hh + 1)],
                                lhsT=kT[p][pr:pr + 64, t0 + k0:t0 + k0 + W],
                                rhs=qT[p][pr:pr + 64, t0 + q0:t0 + q0 + nq],
                                start=True, stop=True,
                                tile_position=(pr, 0))
                    if merged:
                        et = epool.tile([128, 512], BF16, tag="et", name="et")
                        nc.scalar.activation(out=et[0:W, 0:2 * fn],
                                             in_=sts[0][0:W, 0:2 * fn],
                                             func=AF.Exp)
                        ex = epool.tile([128, 512], BF16, tag="ex", name="ex")
                        nc.vector.tensor_mul(
                            ex[0:W, 0:2 * fn], et[0:W, 0:2 * fn],
                            EB[0:W, QB_OFF[qb]:QB_OFF[qb] + 2 * fn])
                        exh = [ex[:, 0:fn], ex[:, fn:2 * fn]]
                    else:
                        for g in range(2):
                            st = sts[g]
                            et = epool.tile([128, 512], BF16, tag="et",
                                            name="et")
                            nc.scalar.activation(out=et[0:W, 0:fn],
                                                 in_=st[0:W, 0:fn], func=AF.Exp)
                            ex = epool.tile([128, 512], BF16, tag="ex",
                                            name="ex")
                            nc.vector.tensor_mul(
                                ex[0:W, 0:fn], et[0:W, 0:fn],
                                EB[0:W,
                                   QB_OFF[qb] + fn * g:QB_OFF[qb] + fn * g + fn])
                            exh.append(ex[:, 0:fn])
                    self.exh[qb] = exh

                def av(self, qb):
                    q0, nq, k0, W = QBLOCKS[qb]
                    exh = self.exh[qb]
                    if self.den is None:
                        self.den = pden.tile([128, 16], F32, tag="den",
                                             name="den")
                    for (pqb, cs, nqp, b0, yt) in PIECES:
                        if pqb != qb:
                            continue
                        if yt not in self.oab:
                            self.oab[yt] = poa.tile([128, 512], F32,
                                                    tag="oab", name="oab")
                        for g in range(2):
                            for hh in range(4):
                                h = 4 * g + hh
                                exs = exh[g][0:W,
                                             nq * hh + cs:nq * hh + cs + nqp]
                                nc.tensor.matmul(
                                    out=self.oab[yt][b0:b0 + nqp,
                                                     64 * h:64 * h + 64],
                                    lhsT=exs,
                                    rhs=self.vwin[qb][0:W, 64 * h:64 * h + 64],
                                    start=True, stop=True,
                                    tile_position=(0, b0))
                                nc.tensor.matmul(
                                    out=self.den[b0:b0 + nqp,
                                                 8 * yt + h:8 * yt + h + 1],
                                    lhsT=exs, rhs=onec[0:W, :],
                                    start=True, stop=True,
                                    tile_position=(0, b0))

                def finish(self, yt):
                    oa = self.oab[yt]
                    rc = rpool.tile([128, 8], F32, tag="rc", name="rc")
                    yf = ypool.tile([128, C], F16, tag="yf", name="yf")
                    nc.vector.reciprocal(rc, self.den[:, 8 * yt:8 * yt + 8])
                    in1 = bass.AP(tensor=rc.tensor, offset=rc.offset,
                                  ap=[rc.ap[0], [1, 8], [0, 64]])
                    nc.vector.scalar_tensor_tensor(
                        out=yf, in0=oa, scalar=1.0, in1=in1,
                        op0=mybir.AluOpType.mult, op1=mybir.AluOpType.mult)
                    nc.sync.dma_start(
                        out=y_ap[self.t0 + 128 * yt:self.t0 + 128 * (yt + 1), :],
                        in_=yf)

            # warm the ACT function table (Exp) during the DMA phase
            dummy = singles.tile([1, 1], F32, tag="dummy")
            nc.scalar.activation(out=dummy, in_=ones_r[0:1, 0:1], func=AF.Exp)

            # ---- software-pipelined emission ----
            # chunk 0 projection up front; afterwards the qk octs of chunk
            # ch feed attention bf=2ch..2ch+1 and are drip-fed as PE filler
            # between scores and AV stages (covering exp latency without
            # clogging the 4-deep PE wait queue).
            from collections import deque
            fillers = deque((ch, oct) for ch in range(1, NCH)
                            for oct in range(8))
            emitted = {0: 8}

            def run_filler(n):
                for _ in range(n):
                    if fillers:
                        ch, oct = fillers.popleft()
                        qk_oct(ch, oct)
                        emitted[ch] = emitted.get(ch, 0) + 1

            def drain_chunk(ch):
                while fillers and fillers[0][0] <= ch:
                    c, oct = fillers.popleft()
                    qk_oct(c, oct)
                    emitted[c] = emitted.get(c, 0) + 1

            for oct in (0, 4, 1, 5, 2, 6, 3, 7):
                qk_oct(0, oct)

            vtiles = {}

            def new_vset(bf):
                vA = vpool.tile([128, 512], BF16, tag="vA", name="vA")
                vC = vpool.tile([128, 512], BF16, tag="vC", name="vC")
                vB = vpool.tile([128, 512], BF16, tag="vB", name="vB")
                vD = vpool.tile([128, 512], BF16, tag="vD", name="vD")
                vtiles[bf] = (vA, vC, vB, vD)
                return vtiles[bf]

            def v_shuffles(bf):
                vA, vC, vB, vD = vtiles[bf]
                nc.sync.dma_start(out=vB[0:64, :], in_=vA[64:128, :])
                nc.sync.dma_start(out=vB[64:128, :], in_=vC[0:64, :])
                nc.sync.dma_start(out=vD[0:96, :], in_=vC[32:128, :])

            vA0, vC0, _, _ = new_vset(0)
            v_half(0, 0, vA0)
            v_half(0, 1, vC0)
            v_shuffles(0)

            for bf in range(BFL - 2):
                if bf % 2 == 0:
                    drain_chunk(bf // 2)   # this chunk's q/k must be ready
                vA, vC, vB, vD = vtiles[bf]
                att = Att(bf, {0: vA, 1: vB, 2: vD})
                att.scores(0)
                nvA, nvC, _, _ = new_vset(bf + 1)
                v_half(bf + 1, 0, nvA)
                att.av(0)
                run_filler(1)
                att.scores(1)
                v_half(bf + 1, 1, vtiles[bf + 1][1])
                v_shuffles(bf + 1)
                att.av(1)
                run_filler(1)
                att.finish(0)
                att.scores(2)
                run_filler(1)
                att.av(2)
                att.finish(1)
                del vtiles[bf]

            # hand-scheduled endgame: no projection filler remains, so the
            # two final bf interleave, with late stages drawn from the idle
            # qkv psum pool to fill exp-latency gaps
            drain_chunk(3)
            b6, b7 = BFL - 2, BFL - 1
            vA, vC, vB, vD = vtiles[b6]
            a6 = Att(b6, {0: vA, 1: vB, 2: vD})
            a6.scores(0)
            nvA, nvC, _, _ = new_vset(b7)
            v_half(b7, 0, nvA)
            a6.av(0)
            a6.scores(1)
            v_half(b7, 1, vtiles[b7][1])
            v_shuffles(b7)
            a6.scores(2, pool=pqkv)
            a6.av(1)
            a6.finish(0)
            vA, vC, vB, vD = vtiles[b7]
            a7 = Att(b7, {0: vA, 1: vB, 2: vD})
            a7.scores(0, pool=pqkv)
            a7.scores(1)
            a6.av(2)
            a6.finish(1)
            a7.scores(2, pool=pqkv)
            a7.av(0)
            a7.av(1)
            a7.finish(0)
            a7.av(2)
            a7.finish(1)

    nc.compile()
    return nc


def _f8rt(a):
    import ml_dtypes
    return np.asarray(a, np.float32).astype(
        ml_dtypes.float8_e4m3).astype(np.float32)


def _tri_terms_w(Wp):
    W1 = _f8rt(Wp)
    W2 = _f8rt(Wp / 4.0)
    W3 = _f8rt(16.0 * (Wp - W1))
    return np.stack([W1, W2, W3]).reshape(3, 2, 2, 128, 512)  # [t3,p2,kt,p,m]


def _tri_split_w(Wp):
    """Wp [512,512] f32 -> packed [128, 6*W_J] fp8 image (j, kt, m)."""
    import ml_dtypes
    Wt = _tri_terms_w(Wp).transpose(3, 0, 1, 2, 4)  # [p, t3, p2, kt, m]
    return np.ascontiguousarray(
        Wt.reshape(128, 6 * W_J).astype(ml_dtypes.float8_e4m3))


def _tri_split_w_m(Wp):
    """Wp [512,512] f32 -> m-major [128, 4*W_MB] fp8 image (mb, j, kt, 128)."""
    import ml_dtypes
    Wt = _tri_terms_w(Wp).reshape(3, 2, 2, 128, 4, 128)
    Wt = Wt.transpose(3, 4, 0, 1, 2, 5)             # [p, mb, t3, p2, kt, m]
    return np.ascontiguousarray(
        Wt.reshape(128, 4 * W_MB).astype(ml_dtypes.float8_e4m3))


def _prep_shared(pe, W_qkv, b_qkv):
    r = np.arange(512)
    head = 2 * (r // 128) + (r % 128) // 64
    cc = r % 64
    qsrc = 192 * head + cc
    ksrc = 192 * head + 64 + cc
    j = np.arange(512)
    vsrc = 192 * (j // 64) + 128 + (j % 64)

    import ml_dtypes
    # q weights quantize UNSCALED (0.125-prescaling would crush them into
    # fp8 subnormals); the 1/8 score scale rides the evacuation copy's
    # per-partition scale (maske/masko).
    WqAv = _tri_split_w_m(np.ascontiguousarray(W_qkv[:, qsrc]))
    WkAv = _tri_split_w_m(np.ascontiguousarray(W_qkv[:, ksrc]))
    WvAv = _tri_split_w(np.ascontiguousarray(W_qkv[:, vsrc]))
    bqv = (b_qkv[qsrc] * 0.125).astype(np.float32).reshape(4, 128).T.copy()
    bkv = b_qkv[ksrc].astype(np.float32).reshape(4, 128).T.copy()

    # BT columns: per qblock, head groups of 4 at 384*g, head-major inside.
    # -80 sentinel: exp(s - 80) underflows bf16 to ~0 for any realistic score.
    BAND_NEG = -80.0
    BTm = np.full((128, 2048), BAND_NEG, dtype=np.float32)
    for qb, (q0, nq, k0, W) in enumerate(QBLOCKS):
        for h in range(NH):
            jj = np.arange(W)[:, None]
            ii = np.arange(nq)[None, :]
            d = (q0 - k0) + ii - jj
            valid = (d >= 0) & (d <= LB)
            idx = np.clip(LB - d, 0, LB)
            blk = np.where(valid, pe[h][idx], BAND_NEG).astype(np.float32)
            off = QB_OFF[qb] + 4 * nq * (h // 4) + nq * (h % 4)
            BTm[0:W, off:off + nq] = blk
    EBm = np.exp(np.minimum(BTm, 60.0)).astype(ml_dtypes.bfloat16)
    bvh = b_qkv[vsrc].astype(np.float32)
    return dict(WqA=WqAv, WkA=WkAv, WvA=WvAv, bq=bqv,
                bk=bkv, EB=EBm,
                onec=np.ones((128, 1), dtype=ml_dtypes.bfloat16),
                ones=np.ones((1, 128), dtype=ml_dtypes.bfloat16)), bvh


def _pack_x(xs):
    """xs [2048, 512] f32 -> [128, 8*XF_S] fp8 sub-chunk-major image."""
    import ml_dtypes
    A = _f8rt(xs)
    Cc = _f8rt(4.0 * (xs - A))
    B = _f8rt(xs / 16.0)
    arr = np.stack([A, Cc, B])                     # [t3, tok, cin]
    arr = arr.reshape(3, 8, 256, 2, 2, 128)        # [t3, s, n, p2, kt, p]
    arr = arr.transpose(5, 1, 0, 3, 4, 2)          # [p, s, t3, p2, kt, n]
    return np.ascontiguousarray(
        arr.reshape(128, 8 * XF_S).astype(ml_dtypes.float8_e4m3))


def kernel(x, pe, W_qkv, b_qkv, num_heads):
    assert int(num_heads) == NH and x.shape == (BF, T, C)
    if "nc" not in _CACHE:
        _CACHE["nc"] = _build_module()
    nc = _CACHE["nc"]

    shared, bvh = _prep_shared(np.asarray(pe, np.float32),
                               np.asarray(W_qkv, np.float32),
                               np.asarray(b_qkv, np.float32))
    in_maps = []
    for c in range(NCORES):
        xs = np.asarray(x[BFL * c:BFL * (c + 1)], np.float32).reshape(TOK, C)
        m = dict(shared)
        m["xF"] = _pack_x(xs)
        in_maps.append(m)
    res = run_bass_kernel_spmd(nc, in_maps, list(range(NCORES)))
    out = np.stack([np.asarray(res.results[c]["y"], np.float32).reshape(BFL, T, C)
                    for c in range(NCORES)], axis=0)
    # v-bias: softmax weights sum to 1, so the bias is a constant per channel
    return (out.reshape(BF, T, C) + bvh[None, None, :]).astype(np.float32)
